# revision 3
# baseline (speedup 1.0000x reference)
"""Trainium2 Bass kernel for an RNN-T style joint network MLP.

  out[b,t,u,o] = tanh(enc[b,t,:] @ W1[:512] + dec[b,u,:] @ W1[512:] + b1) @ W2 + b2

Shapes: enc (8, 256, 512), dec (8, 64, 512), W1 (1024, 1024), b1 (1024,),
W2 (1024, 128), b2 (128,), out (8, 256, 64, 128), all float32.

Sharding: data-parallel over batch — one batch element per NeuronCore, no
collectives.  The kernel is elementwise-bound (16.8M hidden elements per
core need a broadcast-add + tanh), so the work is spread over three engines:
  - ACT: big tanh ops (1 elem/cycle/lane, the pace-setter),
  - DVE: per-(hc,u) tensor_scalar broadcast-adds at 2x, PSUM evacuation,
    and a custom deg-7 odd-polynomial tanh (TANH7_ANT, 8 chained ALU
    slices) that takes over one h-chunk's tanh on most blocks,
  - GPSIMD: stride-0-broadcast tensor_tensor adds for 2-3 h-chunks/block.
PE runs the three GEMMs (enc/dec projections + main (B,T,U,H)x(H,O)).
Per-block assignment tables (GPS_HC7_BLOCKS / DVE_TANH_BLOCKS) balance the
engines; tuned from perfetto traces.
"""

import os
import numpy as np
import ml_dtypes

B, T, U, D, H, O = 8, 256, 64, 512, 1024, 128
NCORES = 8
UB = 4            # u-block size (pipeline granularity)
HC = H // 128     # 8 h-chunks
NB = U // UB      # 16 u-blocks

# deg-7 odd polynomial fit of tanh on [-3.6, 3.6], weight exp(-x^2/4s^2)+0.1
# (s=0.578 = empirical std of the pre-activation). data_rms=8.8e-3.
TANH7_C = (0.9555391354960743, -0.20140714151381847,
           0.023106368613611413, -0.0009152143704840363)

# --- engine assignment tables (tunable) ---
# gpsimd always adds hc5,hc6; also hc7 on these blocks:
GPS_HC7_BLOCKS = frozenset(range(1, 14))
# DVE custom TANH7 handles hc7's tanh on these blocks (else ACT):
DVE_TANH_BLOCKS = frozenset(range(2, 14))

_CACHE = {}
LAST_RESULT = None  # BassKernelResults from the most recent run (for profiling)


def _register_tanh7():
    """Register the TANH7_ANT custom DVE op (documented extension point:
    dve_ops.OPS + name->row map).  Idempotent."""
    import concourse.dve_ops as dve_ops
    from concourse.dve_spec import Spec, Src0, C0, C1, C2, C3, sq, _spill_c3_to_src1

    for op in dve_ops.OPS:
        if op.name == "TANH7_ANT":
            return op

    x = Src0
    s = sq(x)
    body = _spill_c3_to_src1(x * (C0 + s * (C1 + s * (C2 + s * C3))))

    def ref(in0, in1, s0, s1, imm2):
        xf = in0.astype(np.float32)
        c7 = in1.reshape(in1.shape[0], -1)[:, :1].astype(np.float32)
        ss = xf * xf
        return (xf * (s0 + ss * (s1 + ss * (imm2 + ss * c7)))).astype(np.float32)

    op = dve_ops.DveOp(
        "TANH7_ANT",
        Spec(body=body, reference=ref),
        subdim=False,
        uops_sha={"v3": "fabb8cce46cda8f0", "v4": "fc4459a23b42cb8f"},
    )
    dve_ops.OPS.append(op)
    dve_ops.CUSTOM_DVE_SPECS[op.name] = op.spec
    dve_ops._SUB_OPCODE_FOR_NAME[op.name] = (
        dve_ops._CUSTOM_DVE_ROW_BASE + len(dve_ops.OPS) - 1)
    return op


def _build_program():
    from concourse import bacc, tile
    import concourse.mybir as mybir

    TANH7 = _register_tanh7()

    dt = mybir.dt
    f32, bf16 = dt.float32, dt.bfloat16
    Act = mybir.ActivationFunctionType
    Alu = mybir.AluOpType

    nc = bacc.Bacc("TRN2", target_bir_lowering=False, debug=False)

    encT = nc.dram_tensor("encT", [D, T], bf16, kind="ExternalInput").ap()
    decT = nc.dram_tensor("decT", [D, U], bf16, kind="ExternalInput").ap()
    W1 = nc.dram_tensor("W1", [2 * D, H], bf16, kind="ExternalInput").ap()
    W2bf = nc.dram_tensor("W2bf", [H, O], bf16, kind="ExternalInput").ap()
    b1r = nc.dram_tensor("b1r", [128, HC], f32, kind="ExternalInput").ap()
    b2c = nc.dram_tensor("b2c", [O, 1], f32, kind="ExternalInput").ap()
    outT = nc.dram_tensor("outT", [O, U, T], f32, kind="ExternalOutput").ap()

    BW = UB * 2048  # per-block sum/tanh width (hc-major: [hc][u][t])
    HCW = UB * T    # per-(block, hc) width = 1024

    with tile.TileContext(nc) as tc:
        with tc.tile_pool(name="persist", bufs=1) as persist, \
             tc.tile_pool(name="sums", bufs=3) as sums_pool, \
             tc.tile_pool(name="tanhp", bufs=3) as tanh_pool, \
             tc.tile_pool(name="outsb", bufs=3) as out_pool, \
             tc.tile_pool(name="hpsum", bufs=2, space="PSUM") as hpsum_pool, \
             tc.tile_pool(name="psum", bufs=3, space="PSUM") as psum_pool:

            w1_sb = persist.tile([128, 8 * H], bf16, tag="w1")
            encT_sb = persist.tile([128, 4 * T], bf16, tag="encT")
            decT_sb = persist.tile([128, 4 * U], bf16, tag="decT")
            w2_sb = persist.tile([128, HC * O], bf16, tag="w2")
            b1_sb = persist.tile([128, HC], f32, tag="b1")
            b2_sb = persist.tile([128, 1], f32, tag="b2")
            c7_sb = persist.tile([128, 1], f32, tag="c7")
            e_sb = persist.tile([128, HC * T], bf16, tag="eproj")
            bias_sb = persist.tile([128, HC * U], f32, tag="bias")

            # ---- loads: enc-path first (it gates the pipeline head), W1
            # split per 128-row chunk so transfers spread across DMA rings.
            nc.sync.dma_start(encT_sb[:, :].rearrange("p (c t) -> p c t", c=4),
                              encT[:, :].rearrange("(c p) t -> p c t", p=128))
            for c in range(4):
                nc.sync.dma_start(
                    w1_sb[:, c * H:(c + 1) * H],
                    W1[c * 128:(c + 1) * 128, :])
            nc.sync.dma_start(decT_sb[:, :].rearrange("p (c u) -> p c u", c=4),
                              decT[:, :].rearrange("(c p) u -> p c u", p=128))
            for c in range(4):
                nc.sync.dma_start(
                    w1_sb[:, (4 + c) * H:(5 + c) * H],
                    W1[512 + c * 128:512 + (c + 1) * 128, :])
            nc.sync.dma_start(b1_sb[:], b1r[:, :])
            nc.sync.dma_start(b2_sb[:], b2c[:, :])
            nc.sync.dma_start(
                w2_sb[:, :].rearrange("p (c o) -> p c o", c=HC),
                W2bf[:, :].rearrange("(c p) o -> p c o", p=128))
            nc.vector.memset(c7_sb[:], float(TANH7_C[3]))

            # ---- first GEMMs, interleaved per h-chunk so downstream adds can
            # start on hc0 while hc1.. are still multiplying.
            # enc: e_projT[h,t] = sum_d W_enc[d,h]*encT[d,t]   (evac on ACT)
            # dec: bias[h,u] = sum_d W_dec[d,h]*decT[d,u] + b1 (evac on DVE)
            for hc in range(HC):
                pe = hpsum_pool.tile([128, T], f32, tag="ps", name=f"pe{hc}")
                for dc in range(4):
                    nc.tensor.matmul(
                        pe[:],
                        lhsT=w1_sb[:, dc * H + hc * 128: dc * H + hc * 128 + 128],
                        rhs=encT_sb[:, dc * T:(dc + 1) * T],
                        start=(dc == 0), stop=(dc == 3),
                    )
                nc.scalar.activation(e_sb[:, hc * T:(hc + 1) * T], pe[:],
                                     Act.Identity)

                pd = hpsum_pool.tile([128, U], f32, tag="ps", name=f"pd{hc}")
                for dc in range(4):
                    nc.tensor.matmul(
                        pd[:],
                        lhsT=w1_sb[:, (4 + dc) * H + hc * 128: (4 + dc) * H + hc * 128 + 128],
                        rhs=decT_sb[:, dc * U:(dc + 1) * U],
                        start=(dc == 0), stop=(dc == 3),
                    )
                nc.vector.tensor_scalar_add(bias_sb[:, hc * U:(hc + 1) * U],
                                            pd[:], b1_sb[:, hc:hc + 1])

            # ---- main pipeline over u-blocks ----
            # sum/tanh layout per block: [hc][u][t] (hc-major); the main GEMM
            # runs N=512 per u-pair into one half of a 2-bank PSUM tile.
            for blk in range(NB):
                u0 = blk * UB
                gps_hcs = (5, 6, 7) if blk in GPS_HC7_BLOCKS else (5, 6)
                dve_hcs = tuple(h for h in range(HC) if h not in gps_hcs)
                dve_tanh = blk in DVE_TANH_BLOCKS and blk not in (0, NB - 1)

                sum_sb = sums_pool.tile([128, BW], bf16, tag="sum")

                # gpsimd: one broadcast tensor_tensor per (hc, 4u quad)
                for hc in gps_hcs:
                    nc.gpsimd.tensor_tensor(
                        sum_sb[:, hc * HCW:(hc + 1) * HCW].rearrange(
                            "p (u t) -> p u t", u=UB),
                        e_sb[:, None, hc * T:(hc + 1) * T].to_broadcast(
                            (128, UB, T)),
                        bias_sb[:, hc * U + u0: hc * U + u0 + UB, None].to_broadcast(
                            (128, UB, T)),
                        Alu.add,
                    )
                # DVE: per-(hc,u) tensor_scalar adds
                for hc in dve_hcs:
                    for ul in range(UB):
                        nc.vector.tensor_scalar_add(
                            sum_sb[:, hc * HCW + ul * T: hc * HCW + ul * T + T],
                            e_sb[:, hc * T:(hc + 1) * T],
                            bias_sb[:, hc * U + u0 + ul: hc * U + u0 + ul + 1],
                        )

                tanh_sb = tanh_pool.tile([128, BW], bf16, tag="tanh")
                if blk in (0, NB - 1):
                    # quarter-split tanh at the pipeline head/tail: blk0's
                    # first quarter needs only 2 h-chunks of adds (faster
                    # fill); blk15's lets the PE chase quarters (short drain)
                    for q in range(4):
                        nc.scalar.activation(
                            tanh_sb[:, q * BW // 4:(q + 1) * BW // 4],
                            sum_sb[:, q * BW // 4:(q + 1) * BW // 4], Act.Tanh)
                elif dve_tanh:
                    nc.scalar.activation(tanh_sb[:, 0:7 * HCW],
                                         sum_sb[:, 0:7 * HCW], Act.Tanh)
                    nc.vector._custom_dve(
                        TANH7,
                        out=tanh_sb[:, 7 * HCW:8 * HCW],
                        in0=sum_sb[:, 7 * HCW:8 * HCW],
                        in1=c7_sb[:, 0:1],
                        s0=float(TANH7_C[0]), s1=float(TANH7_C[1]),
                        imm2=float(TANH7_C[2]),
                    )
                else:
                    nc.scalar.activation(tanh_sb[:], sum_sb[:], Act.Tanh)

                po = psum_pool.tile([128, 2 * 2 * T], f32, tag="ps",
                                    name=f"po{blk}")
                for hc in range(HC):  # hc outer: W2 chunk stays stationary
                    for p in range(2):
                        nc.tensor.matmul(
                            po[:, p * 2 * T:(p + 1) * 2 * T],
                            lhsT=w2_sb[:, hc * O:(hc + 1) * O],
                            rhs=tanh_sb[:, hc * HCW + p * 2 * T: hc * HCW + (p + 1) * 2 * T],
                            start=(hc == 0), stop=(hc == HC - 1),
                        )

                out_sb = out_pool.tile([128, UB * T], f32, tag="osb")
                if blk == NB - 1:
                    # split the final evac+store so the first pair's DMA
                    # starts as soon as its evac lands
                    for p in range(2):
                        nc.vector.tensor_scalar_add(
                            out_sb[:, p * 2 * T:(p + 1) * 2 * T],
                            po[:, p * 2 * T:(p + 1) * 2 * T], b2_sb[:, 0:1])
                        nc.sync.dma_start(
                            outT[:, u0 + 2 * p:u0 + 2 * (p + 1), :],
                            out_sb[:, p * 2 * T:(p + 1) * 2 * T])
                else:
                    nc.vector.tensor_scalar_add(out_sb[:], po[:],
                                                b2_sb[:, 0:1])
                    nc.sync.dma_start(outT[:, u0:u0 + UB, :], out_sb[:])

    nc.compile()
    return nc


def kernel(encoder_state, decoder_state, W1, b1, W2, b2):
    from concourse.bass_utils import run_bass_kernel_spmd
    global LAST_RESULT

    if "nc" not in _CACHE:
        _CACHE["nc"] = _build_program()
    nc = _CACHE["nc"]

    encoder_state = np.asarray(encoder_state, dtype=np.float32)
    decoder_state = np.asarray(decoder_state, dtype=np.float32)
    W1 = np.asarray(W1, dtype=np.float32)
    b1 = np.asarray(b1, dtype=np.float32)
    W2 = np.asarray(W2, dtype=np.float32)
    b2 = np.asarray(b2, dtype=np.float32)

    bf = ml_dtypes.bfloat16
    W1bf = W1.astype(bf)
    W2bf = W2.astype(bf)
    b1r = np.ascontiguousarray(b1.reshape(HC, 128).T)  # [128, 8]
    b2c = np.ascontiguousarray(b2.reshape(O, 1))

    in_maps = []
    for i in range(NCORES):
        in_maps.append({
            "encT": np.ascontiguousarray(encoder_state[i].T.astype(bf)),  # [512, 256]
            "decT": np.ascontiguousarray(decoder_state[i].T.astype(bf)),  # [512, 64]
            "W1": W1bf,
            "W2bf": W2bf,
            "b1r": b1r,
            "b2c": b2c,
        })

    trace = bool(int(os.environ.get("KERNEL_TRACE", "0")))
    res = run_bass_kernel_spmd(nc, in_maps, list(range(NCORES)), trace=trace)
    LAST_RESULT = res

    # gather: outT[core] is [O, U, T] -> out[b, t, u, o]
    out = np.empty((B, T, U, O), dtype=np.float32)
    for i in range(NCORES):
        out[i] = res.results[i]["outT"].transpose(2, 1, 0)
    return out


# revision 6
# speedup vs baseline: 1.3194x; 1.3194x over previous
"""Trainium2 Bass kernel for an RNN-T style joint network MLP.

  out[b,t,u,o] = tanh(enc[b,t,:] @ W1[:512] + dec[b,u,:] @ W1[512:] + b1) @ W2 + b2

Shapes: enc (8, 256, 512), dec (8, 64, 512), W1 (1024, 1024), b1 (1024,),
W2 (1024, 128), b2 (128,), out (8, 256, 64, 128), all float32.

Sharding: data-parallel over batch — one batch element per NeuronCore, no
collectives.  The kernel is elementwise-bound (16.8M hidden elements per
core need a broadcast-add + tanh), so the work is spread over three engines:
  - ACT: big tanh ops (1 elem/cycle/lane, the pace-setter),
  - DVE: per-(hc,u) tensor_scalar broadcast-adds at 2x, PSUM evacuation,
    and a custom deg-7 odd-polynomial tanh (TANH7_ANT, 8 chained ALU
    slices) that takes over one h-chunk's tanh on most blocks,
  - GPSIMD: stride-0-broadcast tensor_tensor adds for 2-3 h-chunks/block.
PE runs the three GEMMs (enc/dec projections + main (B,T,U,H)x(H,O)).
Per-block assignment tables (GPS_HC7_BLOCKS / DVE_TANH_BLOCKS) balance the
engines; tuned from perfetto traces.
"""

import os
import numpy as np
import ml_dtypes

B, T, U, D, H, O = 8, 256, 64, 512, 1024, 128
NCORES = 8
UB = 4            # u-block size (pipeline granularity)
HC = H // 128     # 8 h-chunks
NB = U // UB      # 16 u-blocks

# deg-7 odd polynomial fit of tanh on [-3.6, 3.6], weight exp(-x^2/4s^2)+0.1
# (s=0.578 = empirical std of the pre-activation). data_rms=8.8e-3.
TANH7_C = (0.9555391354960743, -0.20140714151381847,
           0.023106368613611413, -0.0009152143704840363)

# --- engine assignment tables (tunable) ---
# gpsimd broadcast-adds: DISABLED — gpsimd shares its SBUF port with DVE and
# the two engines serialize (measured: each gpsimd op stalls a concurrent DVE
# op for its full duration), and gpsimd's elementwise rate (1.84 ns/elem) is
# worse than DVE's (0.77), so offloading is strictly negative.
GPS_BASE_HCS = ()
GPS_HC7_BLOCKS = frozenset()
# DVE custom TANH7 handles hc7's tanh on these blocks (else ACT).  With evacs
# shiftable to ACT this trade is slightly negative too; keep off.
DVE_TANH_BLOCKS = frozenset()
# blocks whose PSUM evacuation (+b2) runs on ACT instead of DVE (balance knob)
ACT_EVAC_BLOCKS = frozenset({5, 10})

_CACHE = {}
LAST_RESULT = None  # BassKernelResults from the most recent run (for profiling)


def _register_tanh7():
    """Register the TANH7_ANT custom DVE op (documented extension point:
    dve_ops.OPS + name->row map).  Idempotent."""
    import concourse.dve_ops as dve_ops
    from concourse.dve_spec import Spec, Src0, C0, C1, C2, C3, sq, _spill_c3_to_src1

    for op in dve_ops.OPS:
        if op.name == "TANH7_ANT":
            return op

    x = Src0
    s = sq(x)
    body = _spill_c3_to_src1(x * (C0 + s * (C1 + s * (C2 + s * C3))))

    def ref(in0, in1, s0, s1, imm2):
        xf = in0.astype(np.float32)
        c7 = in1.reshape(in1.shape[0], -1)[:, :1].astype(np.float32)
        ss = xf * xf
        return (xf * (s0 + ss * (s1 + ss * (imm2 + ss * c7)))).astype(np.float32)

    op = dve_ops.DveOp(
        "TANH7_ANT",
        Spec(body=body, reference=ref),
        subdim=False,
        uops_sha={"v3": "fabb8cce46cda8f0", "v4": "fc4459a23b42cb8f"},
    )
    dve_ops.OPS.append(op)
    dve_ops.CUSTOM_DVE_SPECS[op.name] = op.spec
    dve_ops._SUB_OPCODE_FOR_NAME[op.name] = (
        dve_ops._CUSTOM_DVE_ROW_BASE + len(dve_ops.OPS) - 1)
    return op


def _build_program():
    from concourse import bacc, tile
    import concourse.mybir as mybir

    TANH7 = _register_tanh7()

    dt = mybir.dt
    f32, bf16 = dt.float32, dt.bfloat16
    Act = mybir.ActivationFunctionType
    Alu = mybir.AluOpType

    nc = bacc.Bacc("TRN2", target_bir_lowering=False, debug=False)

    encT = nc.dram_tensor("encT", [D, T], bf16, kind="ExternalInput").ap()
    decT = nc.dram_tensor("decT", [D, U], bf16, kind="ExternalInput").ap()
    W1 = nc.dram_tensor("W1", [2 * D, H], bf16, kind="ExternalInput").ap()
    W2bf = nc.dram_tensor("W2bf", [H, O], bf16, kind="ExternalInput").ap()
    b1r = nc.dram_tensor("b1r", [128, HC], f32, kind="ExternalInput").ap()
    b2c = nc.dram_tensor("b2c", [O, 1], f32, kind="ExternalInput").ap()
    outT = nc.dram_tensor("outT", [O, U, T], f32, kind="ExternalOutput").ap()

    BW = UB * 2048  # per-block sum/tanh width (hc-major: [hc][u][t])
    HCW = UB * T    # per-(block, hc) width = 1024

    with tile.TileContext(nc) as tc:
        with tc.tile_pool(name="persist", bufs=1) as persist, \
             tc.tile_pool(name="sums", bufs=3) as sums_pool, \
             tc.tile_pool(name="tanhp", bufs=3) as tanh_pool, \
             tc.tile_pool(name="outsb", bufs=3) as out_pool, \
             tc.tile_pool(name="hpsum", bufs=2, space="PSUM") as hpsum_pool, \
             tc.tile_pool(name="psum", bufs=3, space="PSUM") as psum_pool:

            w1_sb = persist.tile([128, 8 * H], bf16, tag="w1")
            encT_sb = persist.tile([128, 4 * T], bf16, tag="encT")
            decT_sb = persist.tile([128, 4 * U], bf16, tag="decT")
            w2_sb = persist.tile([128, HC * O], bf16, tag="w2")
            b1_sb = persist.tile([128, HC], f32, tag="b1")
            b2_sb = persist.tile([128, 1], f32, tag="b2")
            c7_sb = persist.tile([128, 1], f32, tag="c7")
            e_sb = persist.tile([128, HC * T], bf16, tag="eproj")
            bias_sb = persist.tile([128, HC * U], f32, tag="bias")

            # ---- loads: enc-path first (it gates the pipeline head), W1
            # split per 128-row chunk so transfers spread across DMA rings.
            nc.sync.dma_start(encT_sb[:, :].rearrange("p (c t) -> p c t", c=4),
                              encT[:, :].rearrange("(c p) t -> p c t", p=128))
            for c in range(4):
                nc.sync.dma_start(
                    w1_sb[:, c * H:(c + 1) * H],
                    W1[c * 128:(c + 1) * 128, :])
            nc.sync.dma_start(decT_sb[:, :].rearrange("p (c u) -> p c u", c=4),
                              decT[:, :].rearrange("(c p) u -> p c u", p=128))
            for c in range(4):
                nc.sync.dma_start(
                    w1_sb[:, (4 + c) * H:(5 + c) * H],
                    W1[512 + c * 128:512 + (c + 1) * 128, :])
            nc.sync.dma_start(b1_sb[:], b1r[:, :])
            nc.sync.dma_start(b2_sb[:], b2c[:, :])
            nc.sync.dma_start(
                w2_sb[:, :].rearrange("p (c o) -> p c o", c=HC),
                W2bf[:, :].rearrange("(c p) o -> p c o", p=128))
            nc.vector.memset(c7_sb[:], float(TANH7_C[3]))

            # ---- first GEMMs, interleaved per h-chunk so downstream adds can
            # start on hc0 while hc1.. are still multiplying.
            # enc: e_projT[h,t] = sum_d W_enc[d,h]*encT[d,t]   (evac on ACT)
            # dec: bias[h,u] = sum_d W_dec[d,h]*decT[d,u] + b1 (evac on DVE)
            for hc in range(HC):
                pe = hpsum_pool.tile([128, T], f32, tag="ps", name=f"pe{hc}")
                for dc in range(4):
                    nc.tensor.matmul(
                        pe[:],
                        lhsT=w1_sb[:, dc * H + hc * 128: dc * H + hc * 128 + 128],
                        rhs=encT_sb[:, dc * T:(dc + 1) * T],
                        start=(dc == 0), stop=(dc == 3),
                    )
                nc.scalar.activation(e_sb[:, hc * T:(hc + 1) * T], pe[:],
                                     Act.Identity)

                pd = hpsum_pool.tile([128, U], f32, tag="ps", name=f"pd{hc}")
                for dc in range(4):
                    nc.tensor.matmul(
                        pd[:],
                        lhsT=w1_sb[:, (4 + dc) * H + hc * 128: (4 + dc) * H + hc * 128 + 128],
                        rhs=decT_sb[:, dc * U:(dc + 1) * U],
                        start=(dc == 0), stop=(dc == 3),
                    )
                nc.vector.tensor_scalar_add(bias_sb[:, hc * U:(hc + 1) * U],
                                            pd[:], b1_sb[:, hc:hc + 1])

            # ---- main pipeline over u-blocks ----
            # sum/tanh layout per block: [hc][u][t] (hc-major); the main GEMM
            # runs N=512 per u-pair into one half of a 2-bank PSUM tile.
            for blk in range(NB):
                u0 = blk * UB
                gps_hcs = (GPS_BASE_HCS + ((7,) if blk in GPS_HC7_BLOCKS else ())
                           if GPS_BASE_HCS else ())
                dve_hcs = tuple(h for h in range(HC) if h not in gps_hcs)
                dve_tanh = blk in DVE_TANH_BLOCKS and blk not in (0, NB - 1)

                sum_sb = sums_pool.tile([128, BW], bf16, tag="sum")

                # gpsimd: one broadcast tensor_tensor per (hc, 4u quad)
                for hc in gps_hcs:
                    nc.gpsimd.tensor_tensor(
                        sum_sb[:, hc * HCW:(hc + 1) * HCW].rearrange(
                            "p (u t) -> p u t", u=UB),
                        e_sb[:, None, hc * T:(hc + 1) * T].to_broadcast(
                            (128, UB, T)),
                        bias_sb[:, hc * U + u0: hc * U + u0 + UB, None].to_broadcast(
                            (128, UB, T)),
                        Alu.add,
                    )
                # DVE: per-(hc,u) tensor_scalar adds
                for hc in dve_hcs:
                    for ul in range(UB):
                        nc.vector.tensor_scalar_add(
                            sum_sb[:, hc * HCW + ul * T: hc * HCW + ul * T + T],
                            e_sb[:, hc * T:(hc + 1) * T],
                            bias_sb[:, hc * U + u0 + ul: hc * U + u0 + ul + 1],
                        )

                tanh_sb = tanh_pool.tile([128, BW], bf16, tag="tanh")
                if blk in (0, NB - 1):
                    # quarter-split tanh at the pipeline head/tail: blk0's
                    # first quarter needs only 2 h-chunks of adds (faster
                    # fill); blk15's lets the PE chase quarters (short drain)
                    for q in range(4):
                        nc.scalar.activation(
                            tanh_sb[:, q * BW // 4:(q + 1) * BW // 4],
                            sum_sb[:, q * BW // 4:(q + 1) * BW // 4], Act.Tanh)
                elif dve_tanh:
                    nc.scalar.activation(tanh_sb[:, 0:7 * HCW],
                                         sum_sb[:, 0:7 * HCW], Act.Tanh)
                    nc.vector._custom_dve(
                        TANH7,
                        out=tanh_sb[:, 7 * HCW:8 * HCW],
                        in0=sum_sb[:, 7 * HCW:8 * HCW],
                        in1=c7_sb[:, 0:1],
                        s0=float(TANH7_C[0]), s1=float(TANH7_C[1]),
                        imm2=float(TANH7_C[2]),
                    )
                else:
                    nc.scalar.activation(tanh_sb[:], sum_sb[:], Act.Tanh)

                po = psum_pool.tile([128, 2 * 2 * T], f32, tag="ps",
                                    name=f"po{blk}")
                for hc in range(HC):  # hc outer: W2 chunk stays stationary
                    for p in range(2):
                        nc.tensor.matmul(
                            po[:, p * 2 * T:(p + 1) * 2 * T],
                            lhsT=w2_sb[:, hc * O:(hc + 1) * O],
                            rhs=tanh_sb[:, hc * HCW + p * 2 * T: hc * HCW + (p + 1) * 2 * T],
                            start=(hc == 0), stop=(hc == HC - 1),
                        )

                out_sb = out_pool.tile([128, UB * T], f32, tag="osb")
                if blk == NB - 1:
                    # split the final evac+store so the first pair's DMA
                    # starts as soon as its evac lands
                    for p in range(2):
                        nc.vector.tensor_scalar_add(
                            out_sb[:, p * 2 * T:(p + 1) * 2 * T],
                            po[:, p * 2 * T:(p + 1) * 2 * T], b2_sb[:, 0:1])
                        nc.sync.dma_start(
                            outT[:, u0 + 2 * p:u0 + 2 * (p + 1), :],
                            out_sb[:, p * 2 * T:(p + 1) * 2 * T])
                elif blk in ACT_EVAC_BLOCKS:
                    nc.scalar.activation(out_sb[:], po[:], Act.Identity,
                                         bias=b2_sb[:, 0:1])
                    nc.sync.dma_start(outT[:, u0:u0 + UB, :], out_sb[:])
                else:
                    nc.vector.tensor_scalar_add(out_sb[:], po[:],
                                                b2_sb[:, 0:1])
                    nc.sync.dma_start(outT[:, u0:u0 + UB, :], out_sb[:])

    nc.compile()
    return nc


def kernel(encoder_state, decoder_state, W1, b1, W2, b2):
    from concourse.bass_utils import run_bass_kernel_spmd
    global LAST_RESULT

    if "nc" not in _CACHE:
        _CACHE["nc"] = _build_program()
    nc = _CACHE["nc"]

    encoder_state = np.asarray(encoder_state, dtype=np.float32)
    decoder_state = np.asarray(decoder_state, dtype=np.float32)
    W1 = np.asarray(W1, dtype=np.float32)
    b1 = np.asarray(b1, dtype=np.float32)
    W2 = np.asarray(W2, dtype=np.float32)
    b2 = np.asarray(b2, dtype=np.float32)

    bf = ml_dtypes.bfloat16
    W1bf = W1.astype(bf)
    W2bf = W2.astype(bf)
    b1r = np.ascontiguousarray(b1.reshape(HC, 128).T)  # [128, 8]
    b2c = np.ascontiguousarray(b2.reshape(O, 1))

    in_maps = []
    for i in range(NCORES):
        in_maps.append({
            "encT": np.ascontiguousarray(encoder_state[i].T.astype(bf)),  # [512, 256]
            "decT": np.ascontiguousarray(decoder_state[i].T.astype(bf)),  # [512, 64]
            "W1": W1bf,
            "W2bf": W2bf,
            "b1r": b1r,
            "b2c": b2c,
        })

    trace = bool(int(os.environ.get("KERNEL_TRACE", "0")))
    res = run_bass_kernel_spmd(nc, in_maps, list(range(NCORES)), trace=trace)
    LAST_RESULT = res

    # gather: outT[core] is [O, U, T] -> out[b, t, u, o]
    out = np.empty((B, T, U, O), dtype=np.float32)
    for i in range(NCORES):
        out[i] = res.results[i]["outT"].transpose(2, 1, 0)
    return out


# revision 8
# speedup vs baseline: 1.3382x; 1.0143x over previous
"""Trainium2 Bass kernel for an RNN-T style joint network MLP.

  out[b,t,u,o] = tanh(enc[b,t,:] @ W1[:512] + dec[b,u,:] @ W1[512:] + b1) @ W2 + b2

Shapes: enc (8, 256, 512), dec (8, 64, 512), W1 (1024, 1024), b1 (1024,),
W2 (1024, 128), b2 (128,), out (8, 256, 64, 128), all float32.

Sharding: data-parallel over batch — one batch element per NeuronCore, no
collectives.  The kernel is elementwise-bound: 16.8M hidden elements per core
need a broadcast-add (DVE tensor_scalar, 2x bf16) and a tanh (ACT, 1
elem/cycle/lane).  Steady state balances ACT ~= DVE ~= 119us busy:
  - ACT: big per-block tanh ops + a few PSUM evacuations + head e_proj evacs
  - DVE: all 512 broadcast-adds + most PSUM evacuations (+b2)
  - PE:  enc/dec projections, then the main GEMM (N=512 per u-pair)
Head tricks: dma_start issues spread over idle engine queues (each issue
costs ~0.65us serially on its queue), host-side pre-swizzled input layouts
(2KB-contiguous per partition row -> fewest DMA descriptors), and dummy PE
warm-up matmuls so the HAM clock gate reaches 2.4GHz before the real GEMM.
Tail trick: the last u-block is split in two UB=2 halves so the final
tanh->GEMM->evac->DMA chain is short.

GPSIMD broadcast-adds and a custom deg-7 polynomial tanh on DVE were tried
and rejected: GPSIMD shares its SBUF port with the DVE and the two engines
serialize (measured), and the poly-tanh trade (1.25ns/elem DVE for
0.88ns/elem ACT) is worse than moving evacuations to ACT.
"""

import os
import numpy as np
import ml_dtypes

B, T, U, D, H, O = 8, 256, 64, 512, 1024, 128
NCORES = 8
HC = H // 128     # 8 h-chunks

# u-block sizes (pipeline granularity); last block split for a short tail
UBS = [4] * 15 + [2, 2]
# blocks whose PSUM evacuation (+b2) runs on ACT instead of DVE (balance knob;
# early blocks, where DVE is still ramping the add pipeline)
ACT_EVAC_BLOCKS = frozenset({1, 2})
# h-chunks whose e_proj PSUM evac runs on ACT (rest on DVE)
ACT_EEVAC_HCS = frozenset(range(6))
N_WARMUP_MM = 8   # dummy matmuls to lift the PE HAM clock gate before the GEMM

_CACHE = {}
LAST_RESULT = None  # BassKernelResults from the most recent run (for profiling)


def _build_program():
    from concourse import bacc, tile
    import concourse.mybir as mybir

    dt = mybir.dt
    f32, bf16 = dt.float32, dt.bfloat16
    Act = mybir.ActivationFunctionType

    nc = bacc.Bacc("TRN2", target_bir_lowering=False, debug=False)

    # host-side pre-swizzled layouts: every dram row maps to one partition row
    # with a 2KB contiguous extent (fewest DMA descriptors)
    encTr = nc.dram_tensor("encTr", [128, 4 * T], bf16, kind="ExternalInput").ap()
    decTr = nc.dram_tensor("decTr", [128, 4 * U], bf16, kind="ExternalInput").ap()
    W1 = nc.dram_tensor("W1", [2 * D, H], bf16, kind="ExternalInput").ap()
    W2r = nc.dram_tensor("W2r", [128, HC * O], bf16, kind="ExternalInput").ap()
    b1r = nc.dram_tensor("b1r", [128, HC], f32, kind="ExternalInput").ap()
    b2c = nc.dram_tensor("b2c", [O, 1], f32, kind="ExternalInput").ap()
    outT = nc.dram_tensor("outT", [O, U, T], f32, kind="ExternalOutput").ap()

    with tile.TileContext(nc) as tc:
        with tc.tile_pool(name="persist", bufs=1) as persist, \
             tc.tile_pool(name="sums", bufs=3) as sums_pool, \
             tc.tile_pool(name="tanhp", bufs=3) as tanh_pool, \
             tc.tile_pool(name="outsb", bufs=3) as out_pool, \
             tc.tile_pool(name="hpsum", bufs=2, space="PSUM") as hpsum_pool, \
             tc.tile_pool(name="psum", bufs=3, space="PSUM") as psum_pool:

            w1_sb = persist.tile([128, 8 * H], bf16, tag="w1")
            encT_sb = persist.tile([128, 4 * T], bf16, tag="encT")
            decT_sb = persist.tile([128, 4 * U], bf16, tag="decT")
            w2_sb = persist.tile([128, HC * O], bf16, tag="w2")
            b1_sb = persist.tile([128, HC], f32, tag="b1")
            b2_sb = persist.tile([128, 1], f32, tag="b2")
            e_sb = persist.tile([128, HC * T], bf16, tag="eproj")
            bias_sb = persist.tile([128, HC * U], f32, tag="bias")
            scr_sb = persist.tile([128, 512], bf16, tag="scratch")

            # ---- PE warm-up: dummy matmuls on scratch data keep the PE busy
            # from t~7us so the HAM clock gate is at 2.4GHz when the real
            # GEMM starts (saves ~3us of half-clock matmuls at the head).
            nc.vector.memset(scr_sb[:], 0.0)
            pw = hpsum_pool.tile([128, 512], f32, tag="ps", name="warm")
            for i in range(N_WARMUP_MM):
                nc.tensor.matmul(pw[:], lhsT=scr_sb[:, 0:128], rhs=scr_sb[:],
                                 start=True, stop=True)

            # ---- loads: enc-path first (it gates the pipeline head); issue
            # from three different engine queues (SP/ACT HWDGE + gpsimd
            # SWDGE) so the ~0.65us per-dma_start issue cost is paid in
            # parallel, not serially.  One W1 chunk per dma_start keeps each
            # ring at 128x2KB descriptors and the chunks land in parallel.
            nc.sync.dma_start(encT_sb[:], encTr[:, :])
            for c in range(3):
                nc.sync.dma_start(w1_sb[:, c * H:(c + 1) * H],
                                  W1[c * 128:(c + 1) * 128, :])
            for c in range(3, 6):
                nc.scalar.dma_start(w1_sb[:, c * H:(c + 1) * H],
                                    W1[c * 128:(c + 1) * 128, :])
            nc.gpsimd.dma_start(decT_sb[:], decTr[:, :])
            for c in range(6, 8):
                nc.gpsimd.dma_start(w1_sb[:, c * H:(c + 1) * H],
                                    W1[c * 128:(c + 1) * 128, :])
            nc.gpsimd.dma_start(b1_sb[:], b1r[:, :])
            nc.gpsimd.dma_start(w2_sb[:], W2r[:, :])
            nc.gpsimd.dma_start(b2_sb[:], b2c[:, :])

            # ---- first GEMMs, interleaved per h-chunk so downstream adds can
            # start on hc0 while hc1.. are still multiplying.
            # enc: e_projT[h,t] = sum_d W_enc[d,h]*encT[d,t]
            # dec: bias[h,u] = sum_d W_dec[d,h]*decT[d,u] + b1 (evac on DVE)
            for hc in range(HC):
                pe = hpsum_pool.tile([128, T], f32, tag="ps", name=f"pe{hc}")
                for dc in range(4):
                    nc.tensor.matmul(
                        pe[:],
                        lhsT=w1_sb[:, dc * H + hc * 128: dc * H + hc * 128 + 128],
                        rhs=encT_sb[:, dc * T:(dc + 1) * T],
                        start=(dc == 0), stop=(dc == 3),
                    )
                if hc in ACT_EEVAC_HCS:
                    nc.scalar.activation(e_sb[:, hc * T:(hc + 1) * T], pe[:],
                                         Act.Identity)
                else:
                    nc.vector.tensor_copy(e_sb[:, hc * T:(hc + 1) * T], pe[:])

                pd = hpsum_pool.tile([128, U], f32, tag="ps", name=f"pd{hc}")
                for dc in range(4):
                    nc.tensor.matmul(
                        pd[:],
                        lhsT=w1_sb[:, (4 + dc) * H + hc * 128: (4 + dc) * H + hc * 128 + 128],
                        rhs=decT_sb[:, dc * U:(dc + 1) * U],
                        start=(dc == 0), stop=(dc == 3),
                    )
                nc.vector.tensor_scalar_add(bias_sb[:, hc * U:(hc + 1) * U],
                                            pd[:], b1_sb[:, hc:hc + 1])

            # ---- main pipeline over u-blocks ----
            # sum/tanh layout per block: [hc][u][t] (hc-major); the main GEMM
            # runs N=512 per u-pair into one 1-2 bank PSUM tile.
            u0 = 0
            for blk, ub in enumerate(UBS):
                bw = ub * 2048      # block free width
                hcw = ub * T        # per-(block, hc) width

                sum_sb = sums_pool.tile([128, bw], bf16, tag="sum")
                for hc in range(HC):
                    for ul in range(ub):
                        nc.vector.tensor_scalar_add(
                            sum_sb[:, hc * hcw + ul * T: hc * hcw + ul * T + T],
                            e_sb[:, hc * T:(hc + 1) * T],
                            bias_sb[:, hc * U + u0 + ul: hc * U + u0 + ul + 1],
                        )

                tanh_sb = tanh_pool.tile([128, bw], bf16, tag="tanh")
                if blk in (0, 1) or blk >= len(UBS) - 2:
                    # quarter-split tanh at the pipeline head/tail: a quarter
                    # needs only 2 h-chunks of adds (faster fill), and lets
                    # the PE chase quarters at the drain
                    for q in range(4):
                        nc.scalar.activation(
                            tanh_sb[:, q * bw // 4:(q + 1) * bw // 4],
                            sum_sb[:, q * bw // 4:(q + 1) * bw // 4], Act.Tanh)
                elif blk == 2:
                    for q in range(2):
                        nc.scalar.activation(
                            tanh_sb[:, q * bw // 2:(q + 1) * bw // 2],
                            sum_sb[:, q * bw // 2:(q + 1) * bw // 2], Act.Tanh)
                else:
                    nc.scalar.activation(tanh_sb[:], sum_sb[:], Act.Tanh)

                npair = ub // 2
                po = psum_pool.tile([128, npair * 2 * T], f32, tag="ps",
                                    name=f"po{blk}")
                for hc in range(HC):  # hc outer: W2 chunk stays stationary
                    for p in range(npair):
                        nc.tensor.matmul(
                            po[:, p * 2 * T:(p + 1) * 2 * T],
                            lhsT=w2_sb[:, hc * O:(hc + 1) * O],
                            rhs=tanh_sb[:, hc * hcw + p * 2 * T: hc * hcw + (p + 1) * 2 * T],
                            start=(hc == 0), stop=(hc == HC - 1),
                        )

                out_sb = out_pool.tile([128, ub * T], f32, tag="osb")
                if blk in ACT_EVAC_BLOCKS:
                    nc.scalar.activation(out_sb[:], po[:], Act.Identity,
                                         bias=b2_sb[:, 0:1])
                else:
                    nc.vector.tensor_scalar_add(out_sb[:], po[:],
                                                b2_sb[:, 0:1])
                nc.sync.dma_start(outT[:, u0:u0 + ub, :], out_sb[:])
                u0 += ub

    nc.compile()
    return nc


def _host_inputs(enc_i, dec_i, W1bf, W2bf, b1r, b2c):
    """Per-core input map with pre-swizzled layouts (2KB/partition rows)."""
    bf = ml_dtypes.bfloat16
    # encTr[p, c*T+t] = enc[t, c*128+p]
    encT = np.ascontiguousarray(enc_i.T.astype(bf))          # [512, 256]
    encTr = np.ascontiguousarray(
        encT.reshape(4, 128, T).transpose(1, 0, 2).reshape(128, 4 * T))
    decT = np.ascontiguousarray(dec_i.T.astype(bf))          # [512, 64]
    decTr = np.ascontiguousarray(
        decT.reshape(4, 128, U).transpose(1, 0, 2).reshape(128, 4 * U))
    return {"encTr": encTr, "decTr": decTr, "W1": W1bf, "W2r": None,
            "b1r": b1r, "b2c": b2c}


def kernel(encoder_state, decoder_state, W1, b1, W2, b2):
    from concourse.bass_utils import run_bass_kernel_spmd
    global LAST_RESULT

    if "nc" not in _CACHE:
        _CACHE["nc"] = _build_program()
    nc = _CACHE["nc"]

    encoder_state = np.asarray(encoder_state, dtype=np.float32)
    decoder_state = np.asarray(decoder_state, dtype=np.float32)
    W1 = np.asarray(W1, dtype=np.float32)
    b1 = np.asarray(b1, dtype=np.float32)
    W2 = np.asarray(W2, dtype=np.float32)
    b2 = np.asarray(b2, dtype=np.float32)

    bf = ml_dtypes.bfloat16
    W1bf = W1.astype(bf)
    # W2r[p, c*O+o] = W2[c*128+p, o]
    W2r = np.ascontiguousarray(
        W2.astype(bf).reshape(HC, 128, O).transpose(1, 0, 2).reshape(128, HC * O))
    b1r = np.ascontiguousarray(b1.reshape(HC, 128).T)  # [128, 8]
    b2c = np.ascontiguousarray(b2.reshape(O, 1))

    in_maps = []
    for i in range(NCORES):
        m = _host_inputs(encoder_state[i], decoder_state[i], W1bf, None,
                         b1r, b2c)
        m["W2r"] = W2r
        in_maps.append(m)

    trace = bool(int(os.environ.get("KERNEL_TRACE", "0")))
    res = run_bass_kernel_spmd(nc, in_maps, list(range(NCORES)), trace=trace)
    LAST_RESULT = res

    # gather: outT[core] is [O, U, T] -> out[b, t, u, o]
    out = np.empty((B, T, U, O), dtype=np.float32)
    for i in range(NCORES):
        out[i] = res.results[i]["outT"].transpose(2, 1, 0)
    return out


# revision 16
# speedup vs baseline: 1.3657x; 1.0205x over previous
"""Trainium2 Bass kernel for an RNN-T style joint network MLP.

  out[b,t,u,o] = tanh(enc[b,t,:] @ W1[:512] + dec[b,u,:] @ W1[512:] + b1) @ W2 + b2

Shapes: enc (8, 256, 512), dec (8, 64, 512), W1 (1024, 1024), b1 (1024,),
W2 (1024, 128), b2 (128,), out (8, 256, 64, 128), all float32.

Sharding: data-parallel over batch — one batch element per NeuronCore, no
collectives.  The kernel is elementwise-bound: 16.8M hidden elements per core
need a broadcast-add (DVE tensor_scalar, 2x bf16) and a tanh (ACT, 1
elem/cycle/lane).  Steady state balances ACT ~= DVE ~= 119us busy:
  - ACT: big per-block tanh ops + a few PSUM evacuations + head e_proj evacs
  - DVE: all 512 broadcast-adds + most PSUM evacuations (+b2)
  - PE:  enc/dec projections, then the main GEMM (N=512 per u-pair)
Head tricks: dma_start issues spread over idle engine queues (each issue
costs ~0.65us serially on its queue), host-side pre-swizzled input layouts
(2KB-contiguous per partition row -> fewest DMA descriptors), and dummy PE
warm-up matmuls so the HAM clock gate reaches 2.4GHz before the real GEMM.
Tail trick: the last u-block is split in two UB=2 halves so the final
tanh->GEMM->evac->DMA chain is short.

GPSIMD broadcast-adds and a custom deg-7 polynomial tanh on DVE were tried
and rejected: GPSIMD shares its SBUF port with the DVE and the two engines
serialize (measured), and the poly-tanh trade (1.25ns/elem DVE for
0.88ns/elem ACT) is worse than moving evacuations to ACT.
"""

import os
import numpy as np
import ml_dtypes

B, T, U, D, H, O = 8, 256, 64, 512, 1024, 128
NCORES = 8
HC = H // 128     # 8 h-chunks

# u-block sizes (pipeline granularity); last block split for a short tail
UBS = [4] * 15 + [2, 2]
# blocks whose PSUM evacuation (+b2) runs on ACT instead of DVE (balance knob;
# early blocks, where DVE is still ramping the add pipeline)
ACT_EVAC_BLOCKS = frozenset({1, 2})
# h-chunks whose e_proj PSUM evac runs on ACT (rest on DVE)
ACT_EEVAC_HCS = frozenset(range(5))
N_WARMUP_MM = 4   # dummy matmuls to lift the PE HAM clock gate before the GEMM

_CACHE = {}
LAST_RESULT = None  # BassKernelResults from the most recent run (for profiling)


def _build_program():
    from concourse import bacc, tile
    import concourse.mybir as mybir

    dt = mybir.dt
    f32, bf16 = dt.float32, dt.bfloat16
    Act = mybir.ActivationFunctionType

    nc = bacc.Bacc("TRN2", target_bir_lowering=False, debug=False)

    # host-side pre-swizzled layouts: every dram row maps to one partition row
    # with a 2KB contiguous extent (fewest DMA descriptors)
    encTr = nc.dram_tensor("encTr", [128, 4 * T], bf16, kind="ExternalInput").ap()
    decTr = nc.dram_tensor("decTr", [128, 4 * U], bf16, kind="ExternalInput").ap()
    # W1 host-swizzled hc-major: W1e_r[p, hc*512 + dc*128 + j] = W1[dc*128+p, hc*128+j]
    # so each per-hc dma_start (128KB) unblocks that h-chunk's first GEMM.
    W1e = nc.dram_tensor("W1e", [128, HC * 512], bf16, kind="ExternalInput").ap()
    W1d = nc.dram_tensor("W1d", [128, HC * 512], bf16, kind="ExternalInput").ap()
    W2r = nc.dram_tensor("W2r", [128, HC * O], bf16, kind="ExternalInput").ap()
    b1r = nc.dram_tensor("b1r", [128, HC], f32, kind="ExternalInput").ap()
    b2c = nc.dram_tensor("b2c", [O, 1], f32, kind="ExternalInput").ap()
    outT = nc.dram_tensor("outT", [O, U, T], f32, kind="ExternalOutput").ap()

    with tile.TileContext(nc) as tc:
        with tc.tile_pool(name="persist", bufs=1) as persist, \
             tc.tile_pool(name="sums", bufs=3) as sums_pool, \
             tc.tile_pool(name="tanhp", bufs=3) as tanh_pool, \
             tc.tile_pool(name="outsb", bufs=3) as out_pool, \
             tc.tile_pool(name="hpsum", bufs=2, space="PSUM") as hpsum_pool, \
             tc.tile_pool(name="psum", bufs=3, space="PSUM") as psum_pool:

            w1e_sb = persist.tile([128, HC * 512], bf16, tag="w1e")
            w1d_sb = persist.tile([128, HC * 512], bf16, tag="w1d")
            encT_sb = persist.tile([128, 4 * T], bf16, tag="encT")
            decT_sb = persist.tile([128, 4 * U], bf16, tag="decT")
            w2_sb = persist.tile([128, HC * O], bf16, tag="w2")
            b1_sb = persist.tile([128, HC], f32, tag="b1")
            b2_sb = persist.tile([128, 1], f32, tag="b2")
            e_sb = persist.tile([128, HC * T], bf16, tag="eproj")
            bias_sb = persist.tile([128, HC * U], f32, tag="bias")
            scr_sb = persist.tile([128, 512], bf16, tag="scratch")

            # ---- PE warm-up: dummy matmuls on scratch data keep the PE busy
            # from t~7us so the HAM clock gate is at 2.4GHz when the real
            # GEMM starts (saves ~3us of half-clock matmuls at the head).
            nc.vector.memset(scr_sb[:], 0.0)
            pw = hpsum_pool.tile([128, 512], f32, tag="ps", name="warm")
            for i in range(N_WARMUP_MM):
                nc.tensor.matmul(pw[:], lhsT=scr_sb[:, 0:128], rhs=scr_sb[:],
                                 start=True, stop=True)

            # ---- loads: DMA descriptors spray across all 16 rings, so the
            # head is bandwidth-bound (~2.4MB at ~290GB/s = 8.5us).  W1 is
            # loaded hc-sliced (host-swizzled) so hc0's 256KB lands in ~1us
            # and the first GEMM pipelines with the rest of the load.  Issues
            # come from three engine queues (SP/ACT HWDGE + gpsimd SWDGE) so
            # their ~0.65us per-call issue cost is paid in parallel.
            nc.sync.dma_start(encT_sb[:], encTr[:, :])
            for hc in range(5):
                nc.sync.dma_start(w1e_sb[:, hc * 512:(hc + 1) * 512],
                                  W1e[:, hc * 512:(hc + 1) * 512])
            nc.scalar.dma_start(decT_sb[:], decTr[:, :])
            for hc in range(5):
                nc.scalar.dma_start(w1d_sb[:, hc * 512:(hc + 1) * 512],
                                    W1d[:, hc * 512:(hc + 1) * 512])
            nc.gpsimd.dma_start(b1_sb[:], b1r[:, :])
            nc.gpsimd.dma_start(w2_sb[:], W2r[:, :])
            for hc in range(5, HC):
                nc.gpsimd.dma_start(w1e_sb[:, hc * 512:(hc + 1) * 512],
                                    W1e[:, hc * 512:(hc + 1) * 512])
                nc.gpsimd.dma_start(w1d_sb[:, hc * 512:(hc + 1) * 512],
                                    W1d[:, hc * 512:(hc + 1) * 512])
            nc.gpsimd.dma_start(b2_sb[:], b2c[:, :])

            # ---- first GEMMs, interleaved per h-chunk so downstream adds can
            # start on hc0 while hc1.. are still multiplying.
            # enc: e_projT[h,t] = sum_d W_enc[d,h]*encT[d,t]
            # dec: bias[h,u] = sum_d W_dec[d,h]*decT[d,u] + b1 (evac on DVE)
            for hc in range(HC):
                pe = hpsum_pool.tile([128, T], f32, tag="ps", name=f"pe{hc}")
                for dc in range(4):
                    nc.tensor.matmul(
                        pe[:],
                        lhsT=w1e_sb[:, hc * 512 + dc * 128: hc * 512 + dc * 128 + 128],
                        rhs=encT_sb[:, dc * T:(dc + 1) * T],
                        start=(dc == 0), stop=(dc == 3),
                    )
                if hc in ACT_EEVAC_HCS:
                    nc.scalar.activation(e_sb[:, hc * T:(hc + 1) * T], pe[:],
                                         Act.Identity)
                else:
                    nc.vector.tensor_copy(e_sb[:, hc * T:(hc + 1) * T], pe[:])

                pd = hpsum_pool.tile([128, U], f32, tag="ps", name=f"pd{hc}")
                for dc in range(4):
                    nc.tensor.matmul(
                        pd[:],
                        lhsT=w1d_sb[:, hc * 512 + dc * 128: hc * 512 + dc * 128 + 128],
                        rhs=decT_sb[:, dc * U:(dc + 1) * U],
                        start=(dc == 0), stop=(dc == 3),
                    )
                nc.vector.tensor_scalar_add(bias_sb[:, hc * U:(hc + 1) * U],
                                            pd[:], b1_sb[:, hc:hc + 1])

            # ---- main pipeline over u-blocks ----
            # sum/tanh layout per block: [hc][u][t] (hc-major); the main GEMM
            # runs N=512 per u-pair into one 1-2 bank PSUM tile.
            u0 = 0
            for blk, ub in enumerate(UBS):
                bw = ub * 2048      # block free width
                hcw = ub * T        # per-(block, hc) width

                sum_sb = sums_pool.tile([128, bw], bf16, tag="sum")
                for hc in range(HC):
                    for ul in range(ub):
                        nc.vector.tensor_scalar_add(
                            sum_sb[:, hc * hcw + ul * T: hc * hcw + ul * T + T],
                            e_sb[:, hc * T:(hc + 1) * T],
                            bias_sb[:, hc * U + u0 + ul: hc * U + u0 + ul + 1],
                        )

                tanh_sb = tanh_pool.tile([128, bw], bf16, tag="tanh")
                # split tanh at the pipeline head (a quarter needs only 2
                # h-chunks of adds -> faster fill) and tail (lets the PE
                # chase the drain)
                nsplit = 4 if blk == 0 else 2 if blk in (1, len(UBS) - 2, len(UBS) - 1) else 1
                for q in range(nsplit):
                    nc.scalar.activation(
                        tanh_sb[:, q * bw // nsplit:(q + 1) * bw // nsplit],
                        sum_sb[:, q * bw // nsplit:(q + 1) * bw // nsplit],
                        Act.Tanh)

                npair = ub // 2
                po = psum_pool.tile([128, npair * 2 * T], f32, tag="ps",
                                    name=f"po{blk}")
                for hc in range(HC):  # hc outer: W2 chunk stays stationary
                    for p in range(npair):
                        nc.tensor.matmul(
                            po[:, p * 2 * T:(p + 1) * 2 * T],
                            lhsT=w2_sb[:, hc * O:(hc + 1) * O],
                            rhs=tanh_sb[:, hc * hcw + p * 2 * T: hc * hcw + (p + 1) * 2 * T],
                            start=(hc == 0), stop=(hc == HC - 1),
                        )

                out_sb = out_pool.tile([128, ub * T], f32, tag="osb")
                if blk in ACT_EVAC_BLOCKS:
                    nc.scalar.activation(out_sb[:], po[:], Act.Identity,
                                         bias=b2_sb[:, 0:1])
                else:
                    nc.vector.tensor_scalar_add(out_sb[:], po[:],
                                                b2_sb[:, 0:1])
                nc.sync.dma_start(outT[:, u0:u0 + ub, :], out_sb[:])
                u0 += ub

    nc.compile()
    return nc


def _host_inputs(enc_i, dec_i, b1r, b2c):
    """Per-core input map with pre-swizzled layouts (2KB/partition rows)."""
    bf = ml_dtypes.bfloat16
    # encTr[p, c*T+t] = enc[t, c*128+p]
    encT = np.ascontiguousarray(enc_i.T.astype(bf))          # [512, 256]
    encTr = np.ascontiguousarray(
        encT.reshape(4, 128, T).transpose(1, 0, 2).reshape(128, 4 * T))
    decT = np.ascontiguousarray(dec_i.T.astype(bf))          # [512, 64]
    decTr = np.ascontiguousarray(
        decT.reshape(4, 128, U).transpose(1, 0, 2).reshape(128, 4 * U))
    return {"encTr": encTr, "decTr": decTr, "b1r": b1r, "b2c": b2c}


def _host_weights(W1, W2, bf):
    """W1e/W1d hc-major swizzles + W2r."""
    # W1e[p, hc*512 + dc*128 + j] = W1[dc*128 + p, hc*128 + j]
    We = W1[:D].astype(bf).reshape(4, 128, HC, 128)
    W1e = np.ascontiguousarray(We.transpose(1, 2, 0, 3).reshape(128, HC * 512))
    Wd = W1[D:].astype(bf).reshape(4, 128, HC, 128)
    W1d = np.ascontiguousarray(Wd.transpose(1, 2, 0, 3).reshape(128, HC * 512))
    W2r = np.ascontiguousarray(
        W2.astype(bf).reshape(HC, 128, O).transpose(1, 0, 2).reshape(128, HC * O))
    return W1e, W1d, W2r


def kernel(encoder_state, decoder_state, W1, b1, W2, b2):
    from concourse.bass_utils import run_bass_kernel_spmd
    global LAST_RESULT

    if "nc" not in _CACHE:
        _CACHE["nc"] = _build_program()
    nc = _CACHE["nc"]

    encoder_state = np.asarray(encoder_state, dtype=np.float32)
    decoder_state = np.asarray(decoder_state, dtype=np.float32)
    W1 = np.asarray(W1, dtype=np.float32)
    b1 = np.asarray(b1, dtype=np.float32)
    W2 = np.asarray(W2, dtype=np.float32)
    b2 = np.asarray(b2, dtype=np.float32)

    bf = ml_dtypes.bfloat16
    W1e, W1d, W2r = _host_weights(W1, W2, bf)
    b1r = np.ascontiguousarray(b1.reshape(HC, 128).T)  # [128, 8]
    b2c = np.ascontiguousarray(b2.reshape(O, 1))

    in_maps = []
    for i in range(NCORES):
        m = _host_inputs(encoder_state[i], decoder_state[i], b1r, b2c)
        m.update({"W1e": W1e, "W1d": W1d, "W2r": W2r})
        in_maps.append(m)

    trace = bool(int(os.environ.get("KERNEL_TRACE", "0")))
    res = run_bass_kernel_spmd(nc, in_maps, list(range(NCORES)), trace=trace)
    LAST_RESULT = res

    # gather: outT[core] is [O, U, T] -> out[b, t, u, o]
    out = np.empty((B, T, U, O), dtype=np.float32)
    for i in range(NCORES):
        out[i] = res.results[i]["outT"].transpose(2, 1, 0)
    return out
